# revision 36
# baseline (speedup 1.0000x reference)
"""GCN (3x GCNConv+BN+ReLU, FC+sigmoid) on 8 Trainium2 NeuronCores.

Strategy (node-sharded, graph structure preprocessed on host):
  - Nodes sharded 8-ways (6250/core). Edges partitioned by destination core,
    sorted by destination block (128 dsts), padded to 128-edge chunks.
  - Each core's node range is split into K=5 uneven slices (default
    512/1792/1792/1344/810 rows — small first slice so the collective chain
    launches early, small last slice so the final consuming pass is short).
    Source indices are remapped into K "slice tables" of 8*size rows laid
    out as (src_core * size + row_in_slice) — every slice table fits int16
    gather indices, and each slice is AllGathered INDEPENDENTLY. Slice q's
    AllGather fires as soon as the producing layer's GEMMs cover its rows,
    so the K collectives pipeline back-to-back against both the producing
    layer's tail and the consuming layer's earlier passes:
      pass q of layer L+1 (accumulating slice-q sources for every dst block)
      runs while AllGathers q+1..K-1 are still on the wire. The collective
      chain (2 boundaries x ~395us) is the critical path; phases run under
      it at ~50% engine occupancy.
  - Between passes the PSUM partial for each (block, 256 features) is stashed
    in SBUF as bf16 and re-injected into PSUM with an identity matmul at the
    start of the next pass. The last pass runs the epilogue
    h' = relu((t * s_dst) * a_f + cc_f) with BN folded into a/cc, written
    feature-major as the next layer's GEMM lhsT.
  - Layer 1 exploits linearity (agg(x@W1) == agg(x)@W1): the host ships the
    pre-scaled input tables x_q = s .* x (fp8, identical per core, remapped
    layout), so layer 1 has NO GEMM and NO AllGather — it aggregates raw x
    rows in a single pass and applies W1 per destination block afterwards.
  - dma_gather (SWDGE) pulls fp8 table rows per edge chunk; a one-hot S
    matrix (DVE is_equal against an iota row) turns segment-sum into
    PSUM-accumulated matmuls: t[f,d] += msg.T @ S. Gather calls are emitted
    just-in-time inside the block loops so the Pool SEQ program order tracks
    the data flow and mid-phase collectives are not head-of-line blocked.
  - Next-layer GEMM y = s .* (h @ W) is emitted inline (delayed by GEMM_DLY
    blocks); its output rows are split across K per-slice tensors so each
    AllGather sees a disjoint, fully-written input.
  - Final FC + sigmoid on PE/ACT; output assembled on host.
"""
import os
import sys
sys.path.insert(0, "/opt/trn_rl_repo")

import numpy as np
import ml_dtypes

import concourse.bass as bass
import concourse.tile as tile
from concourse import mybir
from concourse.bass_utils import run_bass_kernel_spmd
from concourse.library_config import mlp as LIB_MLP
from concourse.tile_rust import add_dep_helper

BF16 = ml_dtypes.bfloat16
FP8 = ml_dtypes.float8_e4m3
P = 128
NCORES = 8
BN_EPS = 1e-5
PAD_DST = 200.0        # out-of-range dst id for padding edges

LAST_RESULTS = None    # test harness reads exec_time from here
LAST_NC = None         # built program, for cost-model timing in test.py

N_LAYERS = 3

# sweep knobs (env; defaults = shipped config)
KSPLIT = int(os.environ.get("SW_K", "5"))   # node slices per core (even split)
SIZES_ENV = os.environ.get("SW_SIZES", "512,1792,1792,1344,810")  # uneven slice sizes, comma-separated


def _slice_sizes(n_loc):
    if SIZES_ENV:
        qs = [int(v) for v in SIZES_ENV.split(",")]
        assert sum(qs) == n_loc, (qs, n_loc)
    else:
        assert n_loc % KSPLIT == 0
        qs = [n_loc // KSPLIT] * KSPLIT
    assert all(0 < v <= 4095 for v in qs), qs   # int16 gather-index limit
    qoff = np.concatenate([[0], np.cumsum(qs)]).astype(np.int64)
    return len(qs), np.asarray(qs, np.int64), qoff
GT_BUFS = int(os.environ.get("SW_GLO", "12"))
ST_BUFS = int(os.environ.get("SW_STB", "13"))
FP8_TBL = int(os.environ.get("SW_FP8", "1"))
XP_MOD = int(os.environ.get("SW_XP", "2"))  # expand dstid via ACT for batches with idx % XP_MOD != 0
TPS_BUFS = int(os.environ.get("SW_TPS", "3"))
TW_BUFS = int(os.environ.get("SW_TW", "2"))
YSB_BUFS = int(os.environ.get("SW_YSB", "3"))
N_QUEUES = int(os.environ.get("SW_NQ", "1"))
KB = int(os.environ.get("SW_KB", "14"))
GEMM_DLY = int(os.environ.get("SW_GD", "2"))
G_CALL = int(os.environ.get("SW_GC", "8"))  # chunks (of 128 edges) per dma_gather call
LA = int(os.environ.get("SW_LA", "3"))      # JIT gather lookahead (blocks)


def _split_multiwaits(nc):
    """This walrus build allows one sync-wait per instruction; move extras
    onto preceding same-engine NoOps."""
    n_new = 0
    for fn in nc.m.functions:
        for blk in fn.blocks:
            out = []
            changed = False
            for ins in list(blk.instructions):
                si = ins.sync_info
                if si is not None and len(si.on_wait) > 1:
                    waits = list(si.on_wait)
                    for w in waits[:-1]:
                        n_new += 1
                        out.append(mybir.InstNoOp(
                            name=f"I-mwsplit-{n_new}", engine=ins.engine,
                            sync_info=mybir.SyncInfo(on_wait=[w], on_update=[])))
                    si.on_wait = [waits[-1]]
                    changed = True
                out.append(ins)
            if changed:
                blk.instructions = out


def _prep_host(x, edge_index, n_nodes):
    """Shard + sort + pad the graph. Edges are split per (dst core, dst block)
    into K streams by source slice (row_in_core // Q), with indices remapped
    into the slice-table layout (src_core * Q + row_in_slice)."""
    n_loc = n_nodes // NCORES
    K, qsizes, qoff = _slice_sizes(n_loc)
    n_blk = (n_loc + P - 1) // P

    src = np.concatenate([edge_index[0], np.arange(n_nodes, dtype=np.int64)])
    dst = np.concatenate([edge_index[1], np.arange(n_nodes, dtype=np.int64)])
    deg = np.bincount(dst, minlength=n_nodes).astype(np.float32)
    s = (1.0 / np.sqrt(np.maximum(deg, 1.0))).astype(np.float32)

    per_core = []
    for c in range(NCORES):
        mask = (dst >= c * n_loc) & (dst < (c + 1) * n_loc)
        cs, cd = src[mask], dst[mask] - c * n_loc
        blk = cd // P
        order = np.argsort(blk, kind="stable")
        cs, cd, blk = cs[order], cd[order], blk[order]
        sc, sr = cs // n_loc, cs % n_loc
        sq = np.searchsorted(qoff, sr, side="right") - 1
        sidx = sc * qsizes[sq] + (sr - qoff[sq])   # remapped slice-table row
        lists = [[] for _ in range(K)]
        for b in range(n_blk):
            m = blk == b
            bd = cd[m] - b * P
            bq, bi = sq[m], sidx[m]
            for q in range(K):
                mq = bq == q
                lists[q].append((bi[mq], bd[mq]))
        per_core.append(lists)

    # common per-block chunk counts = max over cores (>=1 for stream 0 so
    # the first pass always opens the PSUM accumulator)
    ncnt = np.zeros((K, n_blk), np.int64)
    for c in range(NCORES):
        for q in range(K):
            for b in range(n_blk):
                ncnt[q, b] = max(ncnt[q, b],
                                 (len(per_core[c][q][b][0]) + P - 1) // P)
    ncnt[0] = np.maximum(ncnt[0], 1)
    NCq = [int(ncnt[q].sum()) for q in range(K)]

    def pack(lists, n_chunks_per_blk, total_chunks):
        """Build gidx [128, total*8] int16 (16-wrap, x8 replicated) and
        dstid [128, total] bf16 for one stream."""
        gsrc = np.zeros(total_chunks * P, np.int64)
        gdst = np.full(total_chunks * P, PAD_DST, np.float32)
        pos = 0
        for b in range(len(n_chunks_per_blk)):
            bs, bd = lists[b]
            n = len(bs)
            cap = int(n_chunks_per_blk[b]) * P
            gsrc[pos:pos + n] = bs
            gdst[pos:pos + n] = bd
            pos += cap
        j = np.arange(total_chunks * P)
        gidx16 = np.zeros((16, total_chunks * 8), np.int16)
        gidx16[j % 16, j // 16] = gsrc
        gidx = np.tile(gidx16, (8, 1))
        dstid = np.zeros((P, total_chunks), dtype=BF16)
        dstid[j % P, j // P] = gdst.astype(BF16)
        return gidx, dstid

    cores = []
    for c in range(NCORES):
        gidxs, dstids = [], []
        for q in range(K):
            gidx, dstid = pack(per_core[c][q], ncnt[q], NCq[q])
            gidxs.append(gidx)
            dstids.append(dstid)

        s_loc = s[c * n_loc:(c + 1) * n_loc]
        s_col = np.zeros((P, n_blk), np.float32)
        for b in range(n_blk):
            nb = min(P, n_loc - b * P)
            s_col[:nb, b] = s_loc[b * P:b * P + nb]
        s_bcast = np.tile(s_loc[None, :], (P, 1)).astype(BF16)

        cores.append(dict(gidxs=gidxs, dstids=dstids,
                          s_col=s_col, s_bcast=s_bcast))
    return cores, ncnt, NCq, n_loc, n_blk, s


def _build(n_nodes, n_loc, n_blk, ncnt, NCq, feat, hid, bfc_val):
    nc = bass.Bass(num_swdge_queues=N_QUEUES,
                   dynamic_dma_scratch_size=max(16384, G_CALL * P * 16))
    dt = mybir.dt
    K, qsizes, qoff = _slice_sizes(n_loc)
    tbl_dt = dt.float8e4 if FP8_TBL else dt.bfloat16

    x_in = [nc.declare_dram_parameter(f"x_q{q}", [NCORES * int(qsizes[q]), feat], tbl_dt, isOutput=False)
            for q in range(K)]
    W_in = [nc.declare_dram_parameter(f"W{i}", [feat if i == 1 else hid, hid], dt.bfloat16, isOutput=False)
            for i in (1, 2, 3)]
    wfc_in = nc.declare_dram_parameter("wfc", [P, 2], dt.bfloat16, isOutput=False)
    aff_in = nc.declare_dram_parameter("aff", [P, 12], dt.float32, isOutput=False)
    s_col_in = nc.declare_dram_parameter("s_col", [P, n_blk], dt.float32, isOutput=False)
    s_bc_in = nc.declare_dram_parameter("s_bcast", [P, n_loc], dt.bfloat16, isOutput=False)
    iota_in = nc.declare_dram_parameter("iota", [P, P], dt.bfloat16, isOutput=False)
    ident_in = nc.declare_dram_parameter("ident", [P, P], dt.float16, isOutput=False)
    gidx_in = [nc.declare_dram_parameter(f"gidx{q}", [P, NCq[q] * 8], dt.int16, isOutput=False)
               for q in range(K)]
    dstid_in = [nc.declare_dram_parameter(f"dstid{q}", [P, NCq[q]], dt.bfloat16, isOutput=False)
                for q in range(K)]
    out_ext = nc.declare_dram_parameter("out", [1, n_loc], dt.float32, isOutput=True)

    y_loc = [nc.dram_tensor(f"y_loc{q}", [int(qsizes[q]), hid], tbl_dt)
             for q in range(K)]
    y_tbl = [nc.dram_tensor(f"y_tbl{q}", [NCORES * int(qsizes[q]), hid], tbl_dt,
                            addr_space="Shared")
             for q in range(K)]

    stq = [np.concatenate([[0], np.cumsum(ncnt[q])]) for q in range(K)]
    # per-stream call/batch granularity: cap the consumption SPAN (in dst
    # blocks) of one call/batch so thin streams don't pin pool buffers
    # across many blocks (single-tag FIFO recycling would deadlock)
    Gq = [max(2, min(G_CALL, int(np.ceil(2.0 * NCq[q] / n_blk))))
          for q in range(K)]
    KBq = [max(3, min(KB, int(np.ceil(3.0 * NCq[q] / n_blk))))
           for q in range(K)]
    callsq = [[(c0, min(Gq[q], NCq[q] - c0)) for c0 in range(0, NCq[q], Gq[q])]
              for q in range(K)]
    batchesq = [[(c0, min(KBq[q], NCq[q] - c0)) for c0 in range(0, NCq[q], KBq[q])]
                for q in range(K)]
    # block whose GEMM completes slice q's rows -> AllGather q launch point
    agmap = {}
    for q in range(K):
        agmap.setdefault(int((qoff[q + 1] - 1) // P), []).append(q)

    with tile.TileContext(nc) as tc:
        with tc.tile_pool(name="const", bufs=1) as cpool, \
             tc.tile_pool(name="ht", bufs=2) as hpool, \
             tc.tile_pool(name="gp", bufs=GT_BUFS) as gpool, \
             tc.tile_pool(name="work", bufs=YSB_BUFS) as wpool, \
             tc.tile_pool(name="ep", bufs=2) as epool, \
             tc.tile_pool(name="stp", bufs=ST_BUFS) as spool, \
             tc.tile_pool(name="dstx", bufs=3) as xpool, \
             tc.tile_pool(name="stash", bufs=1) as stash_pool, \
             tc.tile_pool(name="psy", bufs=2, space="PSUM") as psy, \
             tc.tile_pool(name="pst", bufs=TPS_BUFS, space="PSUM") as pst, \
             tc.tile_pool(name="ptw", bufs=TW_BUFS, space="PSUM") as ptw, \
             tc.tile_pool(name="psf", bufs=1, space="PSUM") as psf:

            lib_inst = nc.gpsimd.load_library(LIB_MLP)

            # to_reg leaks a Pool register per call; cache per distinct count
            _nreg = {}

            def nidx_reg(n):
                if n not in _nreg:
                    _nreg[n] = nc.gpsimd.to_reg(n)
                return _nreg[n]

            # ---- constants ----
            iota = cpool.tile([P, P], dt.bfloat16)
            nc.sync.dma_start(out=iota[:], in_=iota_in[:, :])
            ident = cpool.tile([P, P], dt.float16)
            nc.sync.dma_start(out=ident[:], in_=ident_in[:, :])
            gidx = []
            dstid = []
            for q in range(K):
                g = cpool.tile([P, NCq[q] * 8], dt.int16, tag=f"gidx{q}")
                nc.sync.dma_start(out=g[:], in_=gidx_in[q][:, :])
                gidx.append(g)
                d = cpool.tile([P, NCq[q]], dt.bfloat16, tag=f"dstid{q}")
                nc.sync.dma_start(out=d[:], in_=dstid_in[q][:, :])
                dstid.append(d)
            s_col = cpool.tile([P, n_blk], dt.float32)
            nc.sync.dma_start(out=s_col[:], in_=s_col_in[:, :])
            s_bc = cpool.tile([P, n_loc], dt.bfloat16)
            nc.sync.dma_start(out=s_bc[:], in_=s_bc_in[:, :])
            aff = cpool.tile([P, 12], dt.float32)
            nc.sync.dma_start(out=aff[:], in_=aff_in[:, :])
            wfc = cpool.tile([P, 2], dt.bfloat16)
            nc.sync.dma_start(out=wfc[:], in_=wfc_in[:, :])
            Ws = []
            for i in range(3):
                wlo = cpool.tile([P, hid], dt.bfloat16, tag=f"w{i}lo")
                nc.sync.dma_start(out=wlo[:], in_=W_in[i][0:P, :])
                whi = cpool.tile([P, hid], dt.bfloat16, tag=f"w{i}hi")
                nc.sync.dma_start(out=whi[:], in_=W_in[i][P:2 * P, :])
                Ws.append((wlo, whi))

            # SBUF stash of inter-pass partials: per block a [P, 2*P] bf16
            # slice (both feature halves) at col b * 2 * P.
            stash = stash_pool.tile([P, n_blk * 2 * P], dt.float16)

            h_lo = h_hi = None   # produced by the previous layer's epilogue

            def emit_gather_call(q, call, tables):
                c0, cnt = call
                gt = gpool.tile([P, G_CALL * hid], tbl_dt, tag="gt")
                g = nc.gpsimd.dma_gather(
                    out_ap=gt[:, :cnt * hid].rearrange("p (g f) -> p g f", g=cnt),
                    in_ap=tables[q][:, :],
                    idxs_ap=gidx[q][:, c0 * 8:(c0 + cnt) * 8],
                    num_idxs=cnt * P,
                    num_idxs_reg=nidx_reg(cnt * P),
                    elem_size=hid,
                    queue_num=0,
                )
                add_dep_helper(g.ins, lib_inst.ins, sync=False, reason="lib first")
                return gt

            def emit_compare(q, batch, xp):
                c0, cnt = batch
                st = spool.tile([P, KB * P], dt.bfloat16, tag="st")
                if xp:
                    # materialize the dst-id broadcast on ACT so the DVE
                    # is_equal sees packed last dims (2x DVE mode)
                    tmp = xpool.tile([P, KB * P], dt.bfloat16, tag="dx")
                    nc.scalar.activation(
                        tmp[:, :cnt * P].rearrange("p (c d) -> p c d", c=cnt),
                        dstid[q][:, c0:c0 + cnt, None].to_broadcast([P, cnt, P]),
                        mybir.ActivationFunctionType.Copy, scale=1.0)
                    in0 = tmp[:, :cnt * P].rearrange("p (c d) -> p c d", c=cnt)
                else:
                    in0 = dstid[q][:, c0:c0 + cnt, None].to_broadcast([P, cnt, P])
                nc.vector.tensor_tensor(
                    out=st[:, :cnt * P].rearrange("p (c d) -> p c d", c=cnt),
                    in0=in0,
                    in1=iota[:, None, :].to_broadcast([P, cnt, P]),
                    op=mybir.AluOpType.is_equal,
                )
                return st

            for layer in range(N_LAYERS):
                wlo, whi = Ws[layer]
                tables = x_in if layer == 0 else y_tbl

                # ---- JIT gather emission state (one tile tag; emission
                # order == consumption order) ----
                gt_tiles = [[None] * len(callsq[q]) for q in range(K)]
                if layer == 0:
                    # layer 0 consumes all K streams per block: order calls
                    # globally by (first consuming block, stream) so buffers
                    # recycle in consumption order
                    sched = sorted(
                        (int(np.searchsorted(stq[q], c0, side="right") - 1), q, i)
                        for q in range(K)
                        for i, (c0, cnt) in enumerate(callsq[q]))
                else:
                    # pass-major consumption: stream q is consumed entirely
                    # within pass q
                    sched = [(b, q, i) for q in range(K)
                             for b, i in ((int(np.searchsorted(stq[q], c0, side="right") - 1), i)
                                          for i, (c0, cnt) in enumerate(callsq[q]))]
                sched_pos = [0]

                def ensure(q_need, blk_hi):
                    # emit every scheduled call up to the needed stream
                    # position (stream q_need covering blocks < blk_hi)
                    while sched_pos[0] < len(sched):
                        b, q, i = sched[sched_pos[0]]
                        if layer == 0:
                            if b >= blk_hi:
                                break
                        else:
                            if q > q_need or (q == q_need and b >= blk_hi):
                                break
                        gt_tiles[q][i] = emit_gather_call(q, callsq[q][i], tables)
                        sched_pos[0] += 1

                # ---- compares (no table dependency; emitted up front in
                # consumption order) ----
                st_tiles = [[None] * len(batchesq[q]) for q in range(K)]
                if layer == 0:
                    def lblk(q, c0, cnt):
                        return int(np.searchsorted(stq[q], c0 + cnt - 1,
                                                   side="right") - 1)
                    corder = [(lblk(q, c0, cnt), q, i)
                              for q in range(K)
                              for i, (c0, cnt) in enumerate(batchesq[q])]
                    corder = [(q, i) for _, q, i in sorted(corder)]
                else:
                    corder = [(q, i) for q in range(K)
                              for i in range(len(batchesq[q]))]
                for n_emit, (q, i) in enumerate(corder):
                    xp = (XP_MOD > 0) and (n_emit % XP_MOD != 0)
                    st_tiles[q][i] = emit_compare(q, batchesq[q][i], xp)

                # ---- per-block accumulate + epilogue ----
                h_lo_new = hpool.tile([P, n_loc], dt.bfloat16, tag="h0")
                h_hi_new = hpool.tile([P, n_loc], dt.bfloat16, tag="h1")

                def emit_gemm_block(nlayer, gb, hlo_t, hhi_t):
                    # next layer's GEMM for one block: y = s .* (h @ W),
                    # rows split across the K per-slice tensors
                    gbs = gb * P
                    gnb = min(P, n_loc - gbs)
                    wnlo, wnhi = Ws[nlayer]
                    gps = psy.tile([P, hid], dt.float32, tag="ypsum")
                    nc.tensor.matmul(out=gps[:gnb, :], lhsT=hlo_t[:, gbs:gbs + gnb],
                                     rhs=wnlo[:, :], start=True, stop=False)
                    nc.tensor.matmul(out=gps[:gnb, :], lhsT=hhi_t[:, gbs:gbs + gnb],
                                     rhs=wnhi[:, :], start=False, stop=True)
                    gysb = wpool.tile([P, hid], tbl_dt, tag="ysb")
                    nc.scalar.activation(gysb[:gnb, :], gps[:gnb, :],
                                         mybir.ActivationFunctionType.Copy,
                                         scale=s_col[:gnb, gb:gb + 1])
                    r = gbs
                    while r < gbs + gnb:
                        q = int(np.searchsorted(qoff, r, side="right") - 1)
                        r_hi = min(gbs + gnb, int(qoff[q + 1]))
                        nc.sync.dma_start(
                            out=y_loc[q][r - int(qoff[q]):r_hi - int(qoff[q]), :],
                            in_=gysb[r - gbs:r_hi - gbs, :])
                        r = r_hi

                def emit_ag(q):
                    nc.gpsimd.collective_compute(
                        "AllGather", mybir.AluOpType.bypass,
                        replica_groups=[list(range(NCORES))],
                        ins=[y_loc[q][:, :]], outs=[y_tbl[q][:, :]],
                    )

                def accum_chunks(ps, h, q, c_lo, c_hi, start, stop):
                    n = c_hi - c_lo
                    for i, c in enumerate(range(c_lo, c_hi)):
                        gt = gt_tiles[q][c // Gq[q]]
                        goff = (c % Gq[q]) * hid + h * P
                        st = st_tiles[q][c // KBq[q]]
                        soff = (c % KBq[q]) * P
                        nc.tensor.matmul(
                            out=ps[:, h * P:h * P + P],
                            lhsT=gt[:, goff:goff + P],
                            rhs=st[:, soff:soff + P],
                            start=(start and i == 0),
                            stop=(stop and i == n - 1),
                        )

                def epilogue(ps, col0, bs, nb, h, h_new):
                    tmp = epool.tile([P, P], dt.float32, tag="eptmp")
                    nc.vector.tensor_tensor(out=tmp[:, :nb],
                                            in0=ps[:, col0:col0 + nb],
                                            in1=s_bc[:, bs:bs + nb],
                                            op=mybir.AluOpType.mult)
                    a_ap = aff[:, 4 * layer + h:4 * layer + h + 1]
                    cc_ap = aff[:, 4 * layer + 2 + h:4 * layer + 3 + h]
                    nc.scalar.activation(h_new[:, bs:bs + nb], tmp[:, :nb],
                                         mybir.ActivationFunctionType.Relu,
                                         bias=cc_ap, scale=a_ap)

                gemm_done = [False] * n_blk

                def maybe_gemm(b):
                    # GEMM + AllGather emission while walking the final pass
                    if layer < N_LAYERS - 1 and b >= GEMM_DLY:
                        gb = b - GEMM_DLY
                        if not gemm_done[gb]:
                            gemm_done[gb] = True
                            emit_gemm_block(layer + 1, gb, h_lo_new, h_hi_new)
                            for q in agmap.get(gb, ()):
                                emit_ag(q)

                def flush_gemms():
                    if layer < N_LAYERS - 1:
                        for gb in range(n_blk):
                            if not gemm_done[gb]:
                                gemm_done[gb] = True
                                emit_gemm_block(layer + 1, gb, h_lo_new, h_hi_new)
                                for q in agmap.get(gb, ()):
                                    emit_ag(q)

                if layer == 0:
                    # single pass: all K streams per block, W1 applied after
                    # aggregation (linearity)
                    for b in range(n_blk):
                        ensure(0, min(b + LA, n_blk))
                        bs = b * P
                        nb = min(P, n_loc - bs)
                        # ONE start/stop chain per block: PSUM start=True
                        # arms pending-zero for the whole 2KB zero region, so
                        # a second start inside the same [P, 2*P] tile would
                        # wipe the first half's accumulated data.
                        ps = pst.tile([P, 2 * P], dt.float32, tag="tpsum")
                        nz = [q for q in range(K) if stq[q][b] < stq[q][b + 1]]
                        for h in (0, 1):
                            for j, q in enumerate(nz):
                                accum_chunks(ps, h, q, stq[q][b], stq[q][b + 1],
                                             h == 0 and j == 0,
                                             h == 1 and j == len(nz) - 1)
                        traws = []
                        for h in (0, 1):
                            tr = wpool.tile([P, P], dt.bfloat16, tag=f"traw{h}")
                            nc.scalar.activation(tr[:, :nb], ps[:, h * P:h * P + nb],
                                                 mybir.ActivationFunctionType.Copy,
                                                 scale=1.0)
                            traws.append(tr)
                        for h, h_new in ((0, h_lo_new), (1, h_hi_new)):
                            pw = ptw.tile([P, P], dt.float32, tag="twps")
                            nc.tensor.matmul(out=pw[:, :nb],
                                             lhsT=wlo[:, h * P:(h + 1) * P],
                                             rhs=traws[0][:, :nb],
                                             start=True, stop=False)
                            nc.tensor.matmul(out=pw[:, :nb],
                                             lhsT=whi[:, h * P:(h + 1) * P],
                                             rhs=traws[1][:, :nb],
                                             start=False, stop=True)
                            epilogue(pw, 0, bs, nb, h, h_new)
                        maybe_gemm(b)
                    flush_gemms()
                else:
                    # K passes over all blocks: pass q accumulates stream-q
                    # chunks; intermediate passes stash the partial in SBUF
                    # (bf16) and the next pass re-injects it via an identity
                    # matmul. Pass q runs under AllGathers q+1..K-1 of the
                    # previous boundary. The last pass runs the epilogue and
                    # the next layer's GEMM/AllGathers.
                    for q in range(K):
                        first, last = q == 0, q == K - 1
                        for b in range(n_blk):
                            ensure(q, min(b + LA, n_blk))
                            bs = b * P
                            nb = min(P, n_loc - bs)
                            c_lo, c_hi = stq[q][b], stq[q][b + 1]
                            so = b * 2 * P
                            if first and c_lo == c_hi:
                                continue  # guarded: ncnt[0] >= 1
                            # single start/stop chain across both feature
                            # halves (see the zero-region note above)
                            ps = pst.tile([P, 2 * P], dt.float32, tag="tpsum")
                            if not first:
                                for h in (0, 1):
                                    nc.tensor.matmul(
                                        out=ps[:, h * P:h * P + P],
                                        lhsT=ident[:, :],
                                        rhs=stash[:, so + h * P:so + h * P + P],
                                        start=(h == 0),
                                        stop=(c_lo == c_hi and h == 1))
                            for h in (0, 1):
                                accum_chunks(ps, h, q, c_lo, c_hi,
                                             first and h == 0, h == 1)
                            if not last:
                                nc.scalar.activation(
                                    stash[:, so:so + 2 * P], ps[:, :],
                                    mybir.ActivationFunctionType.Copy,
                                    scale=1.0)
                            else:
                                for h, h_new in ((0, h_lo_new), (1, h_hi_new)):
                                    epilogue(ps, h * P, bs, nb, h, h_new)
                                maybe_gemm(b)
                    flush_gemms()
                h_lo, h_hi = h_lo_new, h_hi_new

            # ---- FC + sigmoid (chunked output, small SBUF footprint) ----
            for t0 in range(0, n_loc, 512):
                w = min(512, n_loc - t0)
                ps = psf.tile([1, 512], dt.float32, tag="fcps")
                nc.tensor.matmul(out=ps[:1, :w], lhsT=wfc[:, 0:1],
                                 rhs=h_lo[:, t0:t0 + w], start=True, stop=False)
                nc.tensor.matmul(out=ps[:1, :w], lhsT=wfc[:, 1:2],
                                 rhs=h_hi[:, t0:t0 + w], start=False, stop=True)
                osb = epool.tile([1, 512], dt.float32, tag="fcosb")
                nc.scalar.activation(osb[:1, :w], ps[:1, :w],
                                     mybir.ActivationFunctionType.Sigmoid,
                                     bias=float(bfc_val), scale=1.0)
                nc.sync.dma_start(out=out_ext[:, t0:t0 + w], in_=osb[:1, :w])

    mybir.codegen_inst_isa_subclasses(nc)
    _split_multiwaits(nc)
    return nc


def kernel(**inputs):
    global LAST_RESULTS, LAST_NC
    x = np.asarray(inputs["x"], dtype=np.float32)
    edge_index = np.asarray(inputs["edge_index"])
    n_nodes, feat = x.shape
    hid = np.asarray(inputs["W1"]).shape[1]

    cores, ncnt, NCq, n_loc, n_blk, s_all = _prep_host(x, edge_index, n_nodes)
    K, qsizes, qoff = _slice_sizes(n_loc)
    TBL_NP = ml_dtypes.float8_e4m3 if FP8_TBL else BF16
    x_tbl = (s_all[:, None] * x).astype(TBL_NP)  # pre-scaled input table
    # remapped slice-table layout: row (c * qsizes[q] + r) of table q holds
    # node c*n_loc + qoff[q] + r
    x3 = x_tbl.reshape(NCORES, n_loc, feat)
    x_q = [np.ascontiguousarray(x3[:, int(qoff[q]):int(qoff[q + 1])]
                                ).reshape(NCORES * int(qsizes[q]), feat)
           for q in range(K)]

    # BN affine folding: z = (agg + b - m) * a + be,  a = g * rsqrt(v + eps)
    aff = np.zeros((P, 12), np.float32)
    for i in (1, 2, 3):
        g = np.asarray(inputs[f"g{i}"], np.float32)
        be = np.asarray(inputs[f"be{i}"], np.float32)
        m = np.asarray(inputs[f"m{i}"], np.float32)
        v = np.asarray(inputs[f"v{i}"], np.float32)
        b = np.asarray(inputs[f"b{i}"], np.float32)
        a = g / np.sqrt(v + BN_EPS)
        cc = (b - m) * a + be
        L = i - 1
        aff[:, 4 * L + 0] = a[0:P]
        aff[:, 4 * L + 1] = a[P:2 * P]
        aff[:, 4 * L + 2] = cc[0:P]
        aff[:, 4 * L + 3] = cc[P:2 * P]

    wfc_np = np.zeros((P, 2), dtype=BF16)
    Wfc = np.asarray(inputs["Wfc"], np.float32)
    wfc_np[:, 0] = Wfc[0:P, 0].astype(BF16)
    wfc_np[:, 1] = Wfc[P:2 * P, 0].astype(BF16)
    bfc_val = float(np.asarray(inputs["bfc"]).reshape(-1)[0])
    iota_np = np.tile(np.arange(P, dtype=np.float32).astype(BF16)[None, :], (P, 1))
    ident_np = np.eye(P, dtype=np.float16)

    nc = _build(n_nodes, n_loc, n_blk, ncnt, NCq, feat, hid, bfc_val)

    in_maps = []
    for c in range(NCORES):
        d = cores[c]
        im = {
            "W1": np.asarray(inputs["W1"], np.float32).astype(BF16),
            "W2": np.asarray(inputs["W2"], np.float32).astype(BF16),
            "W3": np.asarray(inputs["W3"], np.float32).astype(BF16),
            "wfc": wfc_np, "aff": aff,
            "s_col": d["s_col"], "s_bcast": d["s_bcast"],
            "iota": iota_np, "ident": ident_np,
        }
        for q in range(K):
            im[f"x_q{q}"] = x_q[q]
            im[f"gidx{q}"] = d["gidxs"][q]
            im[f"dstid{q}"] = d["dstids"][q]
        in_maps.append(im)

    res = run_bass_kernel_spmd(nc, in_maps, core_ids=list(range(NCORES)))
    LAST_RESULTS = res
    globals()["LAST_NC"] = nc
    out = np.concatenate([res.results[c]["out"].reshape(-1) for c in range(NCORES)])
    return out.reshape(-1, 1).astype(np.float32)
